# revision 14
# baseline (speedup 1.0000x reference)
"""Trainium2 Bass kernel for BinarySphericalQuantizer (nn_BinarySphericalQuantizer_1168231104637).

Full inputs in, full outputs out. Internally: pure data-parallel over 8
NeuronCores (batch 32 -> 4 per core). All heavy per-element work happens
on-device; the host only combines tiny per-core partial sums.

Math notes:
 - zq = sign(z)/sqrt(18), exact via bit ops.
 - The 512-way softmax over the {+-1}^9 codebook factorizes into per-bit
   sigmoids: prob(c) = prod_j sigma(c_j * 4 z_j / sqrt(18)).
   With th = tanh(-2 z / sqrt(18)):  2*sigma(-x) = 1+th, 2*sigma(x) = 1-th.
   Per-sample prob vectors are Kronecker products of (1+th, 1-th) pairs
   (unnormalized by 2^9 per group; folded into the final host-side scale).
 - avg_prob = (1/N) sum_n W16_n (x) W32_n computed as PSUM-accumulated
   matmuls contracting over samples.
 - per-sample entropy: H = softplus(-x) + x*sigma(-x);
   sum softplus(-x) = 18 N ln2 - sum_n ln(prod_d (1-th)), and the inner
   product is two columns of the Kronecker tiles; sum x*sigma(-x) =
   (k/2) sum z(1+th) via a fused scalar_tensor_tensor accumulate.
 - commit = 0.25/N * (sum z^2 - 2c sum|z| + 18 N c^2).
 - indices: exact replication of the reference float path: the products
   ((zq+1)/2)*2^(17-d) are exact in fp32; row sum + round-to-nearest-even
   via the 2^23 magic-number trick (XLA:CPU's astype(int32) rounds).

HW-codegen constraints honored here:
 - TensorScalarPtr/DMA instruction structs have a single sync-wait slot:
   DMA waits are absorbed by InstTensorCopy probes before any STT op.
 - The kernel-tail drain carries one wait per DMA semaphore + engine sems
   and overflows above ~8: keep total DMA instruction count at 5
   (2 z-in, 2 zq-out, 1 packed small-outputs) and build the basis
   constants on-device with iota instead of DMA-ing them.
"""
import os
import sys
import numpy as np
from contextlib import ExitStack

for _p in ("/opt/trn_rl_repo", "/root/.axon_site/_ro/trn_rl_repo"):
    if os.path.isdir(_p) and _p not in sys.path:
        sys.path.insert(0, _p)

import concourse.bass as bass
import concourse.bacc as bacc
import concourse.tile as tile
import concourse.mybir as mybir
from concourse.bass_utils import run_bass_kernel_spmd

f32, i32, bf16 = mybir.dt.float32, mybir.dt.int32, mybir.dt.bfloat16
AF = mybir.ActivationFunctionType
ALU = mybir.AluOpType
AX = mybir.AxisListType

# ---- problem constants (hardcoded) ----
NCORES = 8
B_TOT, H_, W_, D = 32, 64, 64, 18
NSAMP = B_TOT * H_ * W_              # 131072 total samples
NSH = NSAMP // NCORES                # 16384 per core
P = 128                              # partitions
R = NSH // P                         # 128 samples per partition
NC = 2                               # chunks
S = R // NC                          # 64 samples/partition/chunk
E = S * D                            # free elems per chunk

SQ18 = float(np.sqrt(18.0))
C_ = np.float32(1.0 / SQ18)          # 1/sqrt(18)
K_ = 4.0 / SQ18                      # x = K*z
KH = -2.0 / SQ18                     # tanh scale: th = tanh(KH*z)
CBITS = int(np.float32(C_).view(np.int32))
MAGIC = float(np.float32(2.0 ** 23))

# packed small-output layout: [0:16) stats, [16:80) avg psum, [80:208) idx
PK_ST, PK_AVG, PK_IDX, PK_W = 0, 16, 80, 208

_CACHED = {}


def _build_program():
    nc = bacc.Bacc("TRN2", target_bir_lowering=False, debug=False,
                   num_devices=NCORES)
    z_d = nc.dram_tensor("z", [NSH, D], f32, kind="ExternalInput")
    zq_d = nc.dram_tensor("zq", [NSH, D], f32, kind="ExternalOutput")
    pk_d = nc.dram_tensor("pk", [P, PK_W], f32, kind="ExternalOutput")

    # DRAM chunk views: sample n = p*R + c*S + s
    zdr = z_d.ap().rearrange("(p c s) d -> c p (s d)", p=P, c=NC)
    zqr = zq_d.ap().rearrange("(p c s) d -> c p (s d)", p=P, c=NC)

    with tile.TileContext(nc) as tc, ExitStack() as ctx:
        pool = ctx.enter_context(tc.tile_pool(name="main", bufs=NC))
        one = ctx.enter_context(tc.tile_pool(name="one", bufs=1))
        ppool = ctx.enter_context(tc.tile_pool(name="ps", bufs=1, space="PSUM"))

        # basis constants wb[p, d] = 2^(16-d), built on-device with DVE
        # memsets (gpsimd iota would add a Pool-engine wait to the tail
        # drain, which has an 8-wait budget).
        wbt = one.tile([P, D], f32)
        for d in range(D):
            nc.vector.memset(wbt[:, d:d + 1], float(2.0 ** (16 - d)))

        pk = one.tile([P, PK_W], f32)
        nc.vector.memset(pk[:], 0.0)
        st = pk[:, PK_ST:PK_ST + 16]
        # ACT accum targets live outside pk so the pk DMA has a single
        # (DVE) writer engine -> single sync wait (DMA struct limit).
        sqacc = one.tile([P, 4], f32)
        lnacc = one.tile([P, 1], f32)
        qall = one.tile([P, R], f32)
        zqall = one.tile([P, NC * E], f32)
        idxf = one.tile([P, R], f32)
        ps = ppool.tile([32, 64], f32)

        for ch in range(NC):
            zt = pool.tile([P, E], f32, tag="z")
            nc.sync.dma_start(zt[:], zdr[ch])
            zp = pool.tile([P, 4], f32, tag="zp")
            nc.vector.tensor_copy(zp[:], zt[:, 0:4])  # DVE wait absorber

            # ---- ACT: tanh, square-accumulate ----
            th = pool.tile([P, E], f32, tag="th")
            nc.scalar.activation(th[:], zt[:], AF.Tanh, scale=KH)
            sq = pool.tile([P, E], f32, tag="sq")
            nc.scalar.activation(sq[:], zt[:], AF.Square,
                                 accum_out=sqacc[:, ch:ch + 1])

            # ---- zq (sign bit trick) ----
            zq = zqall[:, ch * E:(ch + 1) * E]
            nc.vector.tensor_scalar(zq.bitcast(i32), zt[:].bitcast(i32),
                                    -0x80000000, CBITS,
                                    op0=ALU.bitwise_and, op1=ALU.bitwise_or)

            # ---- abs + z(1+th) accumulates ----
            ab = pool.tile([P, E], f32, tag="ab")
            nc.vector.scalar_tensor_tensor(ab[:], zt[:], -1.0, zt[:],
                                           op0=ALU.mult, op1=ALU.max,
                                           accum_out=st[:, 4 + ch:5 + ch])
            zth = pool.tile([P, E], f32, tag="zth")
            nc.vector.scalar_tensor_tensor(zth[:], th[:], 1.0, zt[:],
                                           op0=ALU.add, op1=ALU.mult,
                                           accum_out=st[:, 8 + ch:9 + ch])

            # ---- indices: m4 = (zq+1)*wb (exact), row sum ----
            m4 = pool.tile([P, E], f32, tag="m4")
            wb_b = wbt[:].unsqueeze(1).broadcast_to([P, S, D])
            nc.vector.scalar_tensor_tensor(
                m4[:].rearrange("p (s d) -> p s d", d=D),
                zq.rearrange("p (s d) -> p s d", d=D),
                1.0, wb_b, op0=ALU.add, op1=ALU.mult)
            nc.vector.tensor_reduce(
                idxf[:, ch * S:(ch + 1) * S],
                m4[:].rearrange("p (s d) -> p s d", d=D),
                axis=AX.X, op=ALU.add)

            # ---- pair planes: pf=1+th, pm=1-th (bf16) ----
            PB = pool.tile([P, 2 * E], bf16, tag="pb")
            nc.vector.tensor_scalar(PB[:, 0:E], th[:], 1.0, None, op0=ALU.add)
            nc.vector.tensor_scalar(PB[:, E:2 * E], th[:], -1.0, 1.0,
                                    op0=ALU.mult, op1=ALU.add)
            pb = PB[:].rearrange("p (b s d) -> p b s d", b=2, s=S)

            def fac(d):  # -> [P, S, 2]
                return pb[:, :, :, d].transpose([0, 2, 1])

            def kron(out_ap, a_ap, b_ap, na, nb):
                ao = a_ap.unsqueeze(3).broadcast_to([P, S, na, nb])
                bo = b_ap.unsqueeze(2).broadcast_to([P, S, na, nb])
                nc.vector.tensor_tensor(out_ap, ao, bo, op=ALU.mult)

            # ---- Kronecker trees for both groups ----
            # layout (s, g, a): per-slice matmul operands are contiguous
            w16 = pool.tile([P, S * 2 * 16], bf16, tag="w16")
            w32 = pool.tile([P, S * 2 * 32], bf16, tag="w32")
            w16r = w16[:].rearrange("p (s g a) -> p g s a", g=2, a=16)
            w32r = w32[:].rearrange("p (s g a) -> p g s a", g=2, a=32)
            w16m = w16[:].rearrange("p (s c) -> p s c", c=32)
            w32m = w32[:].rearrange("p (s c) -> p s c", c=64)
            for g in range(2):
                o = 9 * g
                la = pool.tile([P, S * 4], bf16, tag=f"la{g}")
                kron(la[:].rearrange("p (s a b) -> p s a b", a=2, b=2),
                     fac(o + 0), fac(o + 1), 2, 2)
                lb = pool.tile([P, S * 4], bf16, tag=f"lb{g}")
                kron(lb[:].rearrange("p (s a b) -> p s a b", a=2, b=2),
                     fac(o + 2), fac(o + 3), 2, 2)
                kron(w16r[:, g],
                     la[:].rearrange("p (s a) -> p s a", a=4),
                     lb[:].rearrange("p (s a) -> p s a", a=4), 4, 4)
                lc = pool.tile([P, S * 4], bf16, tag=f"lc{g}")
                kron(lc[:].rearrange("p (s a b) -> p s a b", a=2, b=2),
                     fac(o + 4), fac(o + 5), 2, 2)
                ld = pool.tile([P, S * 4], bf16, tag=f"ld{g}")
                kron(ld[:].rearrange("p (s a b) -> p s a b", a=2, b=2),
                     fac(o + 6), fac(o + 7), 2, 2)
                l2 = pool.tile([P, S * 16], bf16, tag=f"l2{g}")
                kron(l2[:].rearrange("p (s a b) -> p s a b", a=4, b=4),
                     lc[:].rearrange("p (s a) -> p s a", a=4),
                     ld[:].rearrange("p (s a) -> p s a", a=4), 4, 4)
                kron(w32r[:, g],
                     l2[:].rearrange("p (s a) -> p s a", a=16),
                     fac(o + 8), 16, 2)

            # ---- q = prod of all-(+1) columns (pm products) ----
            q2 = pool.tile([P, 2 * S], f32, tag="q2")
            nc.vector.tensor_tensor(
                q2[:].rearrange("p (g s) -> p g s", g=2),
                w16r[:, :, :, 15], w32r[:, :, :, 31], op=ALU.mult)
            nc.vector.tensor_tensor(
                qall[:, ch * S:(ch + 1) * S],
                q2[:, 0:S], q2[:, S:2 * S], op=ALU.mult)

            # ---- avg_prob matmuls: psum[32,64] += W16s^T @ W32s ----
            for s in range(S):
                first = (ch == 0 and s == 0)
                last = (ch == NC - 1 and s == S - 1)
                nc.tensor.matmul(ps[:], w16m[:, s, :], w32m[:, s, :],
                                 start=first, stop=last)

        # ---- tail: ln(q) accumulate, idx round+convert, pack, DMA ----
        lnq = one.tile([P, R], f32)
        nc.scalar.activation(lnq[:], qall[:], AF.Ln,
                             accum_out=lnacc[:])
        nc.vector.tensor_copy(st[:, 0:NC], sqacc[:, 0:NC])
        nc.vector.tensor_copy(st[:, 12:13], lnacc[:])
        idxn = one.tile([P, R], f32)
        nc.vector.tensor_scalar(idxn[:], idxf[:], MAGIC, MAGIC,
                                op0=ALU.add, op1=ALU.subtract)
        nc.vector.tensor_copy(pk[:, PK_IDX:PK_IDX + R].bitcast(i32), idxn[:])
        nc.vector.tensor_copy(pk[0:32, PK_AVG:PK_AVG + 64], ps[:])
        nc.gpsimd.dma_start(pk_d.ap(), pk[:])
        nc.gpsimd.dma_start(
            zq_d.ap().rearrange("(p r) d -> p (r d)", p=P), zqall[:])

    nc.finalize()
    return nc


def get_program():
    if "nc" not in _CACHED:
        _CACHED["nc"] = _build_program()
    return _CACHED["nc"]


def kernel(z: np.ndarray, _trace: bool = False):
    z = np.ascontiguousarray(np.asarray(z, dtype=np.float32))
    assert z.shape == (B_TOT, H_, W_, D), z.shape
    nc = get_program()

    zs = z.reshape(NCORES, NSH, D)
    in_maps = [{"z": np.ascontiguousarray(zs[c])} for c in range(NCORES)]

    res = run_bass_kernel_spmd(nc, in_maps, list(range(NCORES)),
                               trace=_trace)
    outs = res.results

    zq = np.concatenate([outs[c]["zq"].reshape(NSH, D)
                         for c in range(NCORES)], axis=0)
    zq = zq.reshape(B_TOT, H_, W_, D)

    pks = np.stack([np.asarray(outs[c]["pk"]).reshape(P, PK_W)
                    for c in range(NCORES)])
    indices = np.concatenate(
        [pks[c, :, PK_IDX:PK_IDX + R].view(np.float32)
         .view(np.int32).reshape(-1) for c in range(NCORES)])
    indices = indices.reshape(B_TOT, H_, W_).astype(np.int32)

    st = pks[:, :, PK_ST:PK_ST + 16].astype(np.float64)
    avg = pks[:, 0:32, PK_AVG:PK_AVG + 64].astype(np.float64)

    N = float(NSAMP)
    sum_z2 = st[:, :, 0:4].sum()
    sum_abs = st[:, :, 4:8].sum()
    sum_zth = st[:, :, 8:12].sum()
    sum_lnq = st[:, :, 12].sum()

    c = float(C_)
    k = float(K_)
    commit = 0.25 * (sum_z2 - 2.0 * c * sum_abs + D * N * c * c) / N
    sum_sp = D * N * np.log(2.0) - sum_lnq
    sum_h2 = 0.5 * k * sum_zth
    pse = (sum_sp + sum_h2) / N

    M = avg.sum(axis=0)                       # [32, 64]
    avg_prob = np.empty((2, 512), np.float64)
    avg_prob[0] = (M[0:16, 0:32] / (512.0 * N)).reshape(-1)
    avg_prob[1] = (M[16:32, 32:64] / (512.0 * N)).reshape(-1)
    cb_entropy = -(avg_prob * np.log(avg_prob + 1e-8)).sum()

    loss = commit + (1.0 * pse - 1.0 * cb_entropy)

    out = (zq.astype(np.float32),
           np.float32(loss),
           np.float32(cb_entropy),
           indices,
           avg_prob.astype(np.float32))
    if _trace:
        return out, res
    return out
